# revision 3
# baseline (speedup 1.0000x reference)
"""Trainium2 Bass kernel for nn_BC_5274219839877.

Computes, for b=64, n_v=128, n_q=32, d_v=2048, d_q=1024, K=3072, H=8:
    v_ = relu((v_g/||v_w||) * v @ v_w^T + v_b)        [b, n_v, K]
    q_ = relu((q_g/||q_w||) * q @ q_w^T + q_b)        [b, n_q, K]
    out[b,h,i,j] = sum_k hm[h,k] v_[b,i,k] q_[b,j,k] + h_bias[h]

Sharding: data-parallel over batch across 8 NeuronCores (8 batches/core),
weights replicated. All matmul operands are bf16 (converted host-side):
half the HBM traffic of f32, fast weight loads on the PE, and PSUM still
accumulates in f32 (rel err ~5e-3 << 2e-2 gate).

Structure per core (fused, nothing round-trips to DRAM):
  phase Q : all 24 k-blocks of the q-side projection -> qk_all in SBUF
  phase V0: v-side projection for batches 0-3 + bhvq contraction
  phase V1: same for batches 4-7
Stage-3 accumulates over all 24 k-blocks directly in PSUM: each [128,512]
bank holds two batches' [128,256] accumulators. The first matmul of the
even batch uses start=True (clears has_written for the whole bank); the
odd batch's first matmul uses start=False and lands on cleared bits, so
it overwrites -- both then accumulate for the remaining 23 blocks. This
removes ~190 per-block DVE adds from the steady state.
"""

import numpy as np

import concourse.bass as bass
import concourse.tile as tile
from concourse import bacc, mybir
from concourse.bass_utils import run_bass_kernel_spmd

F32 = mybir.dt.float32
BF16 = mybir.dt.bfloat16

N_CORES = 8
B = 64
B_LOC = B // N_CORES       # 8 batches per core
NV = 128
NQ = 32
DV = 2048
DQ = 1024
K = 3072
H = 8

KB = 128                   # k-block size (PSUM partition dim)
NKB = K // KB              # 24 k-blocks
TV = DV // 128             # 16 d-tiles (v side)
TQ = DQ // 128             # 8 d-tiles (q side)
MV = B_LOC * NV            # 1024 (m = (batch, i))
MQ = B_LOC * NQ            # 256  (m = (batch, j))
MH = MV // 2               # 512: m-half (4 batches) per pass
HN = H * NQ                # 256 output cols per batch

WARM_N = 12                # HAM warmup matmuls

_CACHE = {}


def _build_program():
    nc = bacc.Bacc("TRN2", target_bir_lowering=False, debug=False,
                   num_devices=N_CORES)

    qt_d = nc.dram_tensor("qt", [128, TQ * MQ], BF16, kind="ExternalInput")
    vt_d = nc.dram_tensor("vt", [2, 128, TV * MH], BF16,
                          kind="ExternalInput")
    wq_d = nc.dram_tensor("wq", [NKB // 2, 128, 2 * TQ * KB], BF16,
                          kind="ExternalInput")
    wv_d = nc.dram_tensor("wv", [NKB // 2, 128, 2 * TV * KB], BF16,
                          kind="ExternalInput")
    msb_d = nc.dram_tensor("msb", [128, NKB * H], F32, kind="ExternalInput")
    vb_d = nc.dram_tensor("vb", [128, NKB], F32, kind="ExternalInput")
    qb_d = nc.dram_tensor("qb", [128, NKB], F32, kind="ExternalInput")
    bias_d = nc.dram_tensor("bias", [128, HN], F32, kind="ExternalInput")
    ssb_d = nc.dram_tensor("ssb", [128, 2], F32, kind="ExternalInput")
    out_d = nc.dram_tensor("out", [2, 128, 4 * HN], BF16,
                           kind="ExternalOutput")

    relu = mybir.ActivationFunctionType.Relu

    # fixed SBUF allocations
    msb = nc.alloc_sbuf_tensor("msb_s", [128, NKB * H], F32).ap()
    vb = nc.alloc_sbuf_tensor("vb_s", [128, NKB], F32).ap()
    qb = nc.alloc_sbuf_tensor("qb_s", [128, NKB], F32).ap()
    biasb = nc.alloc_sbuf_tensor("bias_s", [128, HN], F32).ap()
    ssb = nc.alloc_sbuf_tensor("ssb_s", [128, 2], F32).ap()
    warm = nc.alloc_sbuf_tensor("warm", [128, 256], BF16).ap()

    qt = nc.alloc_sbuf_tensor("qt_s", [128, TQ * MQ], BF16).ap()
    vt = nc.alloc_sbuf_tensor("vt_s", [128, 2 * TV * MH], BF16).ap()
    wq_s = [nc.alloc_sbuf_tensor(f"wqs{i}", [128, 2 * TQ * KB], BF16).ap()
            for i in range(3)]
    wv_all = nc.alloc_sbuf_tensor("wv_all", [128, NKB * TV * KB], BF16).ap()
    qk_all = nc.alloc_sbuf_tensor("qk_all", [128, NKB * MQ], BF16).ap()
    qx = [nc.alloc_sbuf_tensor(f"qx{i}", [128, 4 * HN], BF16).ap()
          for i in range(3)]
    vk = [nc.alloc_sbuf_tensor(f"vk{i}", [128, MH], BF16).ap()
          for i in range(3)]
    outs = nc.alloc_sbuf_tensor("outs", [128, B_LOC * HN], BF16).ap()

    psv = [nc.alloc_psum_tensor(f"psv{i}", [128, 512], F32).ap()
           for i in range(2)]
    psq = [nc.alloc_psum_tensor(f"psq{i}", [128, MQ], F32).ap()
           for i in range(2)]
    ps3 = [nc.alloc_psum_tensor(f"ps3{i}", [128, 512], F32).ap()
           for i in range(4)]

    with tile.TileContext(nc) as tc:
        # small persistent consts on the gpsimd queue
        nc.gpsimd.memset(warm, 0.0)
        nc.gpsimd.dma_start(msb, msb_d.ap())
        nc.gpsimd.dma_start(vb, vb_d.ap())
        nc.gpsimd.dma_start(qb, qb_d.ap())
        nc.gpsimd.dma_start(biasb, bias_d.ap())
        nc.gpsimd.dma_start(ssb, ssb_d.ap())

        # sync-queue transfer order is the pacing plan: qt, then wq
        # chunks (feed phase Q), then vt half 0 + wv chunks (feed V0),
        # vt half 1 late (only V1 needs it).
        nc.sync.dma_start(qt, qt_d.ap())
        nc.sync.dma_start(wq_s[0], wq_d[0])
        nc.sync.dma_start(wq_s[1], wq_d[1])

        # HAM pre-warm while the first DMAs stream
        for _ in range(WARM_N):
            nc.tensor.matmul(psv[0][:, :256], warm[:, :128], warm,
                             start=True, stop=True)

        # ---- phase Q: q-side projection, all k-blocks ----
        for kb in range(NKB):
            c = kb // 2
            if kb % 2 == 0 and c + 2 < NKB // 2:
                nc.sync.dma_start(wq_s[(c + 2) % 3], wq_d[c + 2])
            wqb = wq_s[c % 3]
            off = (kb % 2) * TQ * KB
            ps = psq[kb % 2]
            for t in range(TQ):
                nc.tensor.matmul(
                    ps,
                    wqb[:, off + t * KB:off + (t + 1) * KB],
                    qt[:, t * MQ:(t + 1) * MQ],
                    start=(t == 0), stop=(t == TQ - 1))
            nc.scalar.activation(qk_all[:, kb * MQ:(kb + 1) * MQ], ps, relu,
                                 bias=qb[:, kb:kb + 1], scale=ssb[:, 1:2])
            if kb == 18:
                # all wq queued; now queue V0's data behind it
                nc.sync.dma_start(vt[:, :TV * MH], vt_d[0])
                nc.sync.dma_start(wv_all[:, :2 * TV * KB], wv_d[0])
                nc.sync.dma_start(wv_all[:, 2 * TV * KB:4 * TV * KB],
                                  wv_d[1])

        def make_qx(kb, half):
            # qx[k, (b,h,j)] = hm[h,k] * qk[k, (b,j)] for this m-half's
            # 4 batches
            qxb = qx[kb % 3]
            qx4 = qxb.rearrange("p (b h j) -> p b h j", b=4, h=H)
            qk3 = qk_all[:, kb * MQ + half * 128:
                         kb * MQ + half * 128 + 128].rearrange(
                "p (b j) -> p b j", b=4)
            for h in range(H):
                nc.vector.tensor_scalar_mul(
                    qx4[:, :, h, :], qk3,
                    msb[:, kb * H + h:kb * H + h + 1])

        def stage3(kb, half):
            # out[b][i, (h,j)] += vk[:, lb].T @ qx[:, lb]; accumulates
            # in PSUM across all k-blocks (see module docstring)
            vkb = vk[kb % 3]
            qxb = qx[kb % 3]
            for lb in range(4):
                b_ = half * 4 + lb
                bank = ps3[b_ // 2]
                col = (b_ % 2) * HN
                nc.tensor.matmul(
                    bank[:, col:col + HN],
                    vkb[:, lb * NV:(lb + 1) * NV],
                    qxb[:, lb * HN:(lb + 1) * HN],
                    start=(kb == 0 and b_ % 2 == 0),
                    stop=(kb == NKB - 1),
                    skip_group_check=True)

        def drain(half):
            for lb in range(4):
                b_ = half * 4 + lb
                bank = ps3[b_ // 2]
                col = (b_ % 2) * HN
                nc.vector.tensor_add(
                    outs[:, b_ * HN:(b_ + 1) * HN],
                    bank[:, col:col + HN], biasb)
            nc.sync.dma_start(
                out_d[half],
                outs[:, half * 4 * HN:(half + 1) * 4 * HN])

        # ---- phases V0/V1: v-side projection + bhvq contraction ----
        for half in range(2):
            for kb in range(NKB):
                c = kb // 2
                if half == 0 and kb % 2 == 0 and c + 2 < NKB // 2:
                    nc.sync.dma_start(
                        wv_all[:, (c + 2) * 2 * TV * KB:
                               (c + 3) * 2 * TV * KB],
                        wv_d[c + 2])
                if half == 0 and kb == 6:
                    nc.sync.dma_start(vt[:, TV * MH:], vt_d[1])
                ps = psv[kb % 2]
                for t in range(TV):
                    nc.tensor.matmul(
                        ps,
                        wv_all[:, kb * TV * KB + t * KB:
                               kb * TV * KB + (t + 1) * KB],
                        vt[:, half * TV * MH + t * MH:
                           half * TV * MH + t * MH + MH],
                        start=(t == 0), stop=(t == TV - 1))
                nc.scalar.activation(
                    vk[kb % 3], ps, relu,
                    bias=vb[:, kb:kb + 1], scale=ssb[:, 0:1])
                make_qx(kb, half)
                if kb >= 1:
                    stage3(kb - 1, half)
            stage3(NKB - 1, half)
            drain(half)

    nc.compile()
    return nc


def _prep_host(inputs):
    bf16 = mybir.dt.np(BF16)
    v = np.asarray(inputs["v"], dtype=np.float32)
    q = np.asarray(inputs["q"], dtype=np.float32)
    v_w = np.asarray(inputs["v_w"], dtype=np.float32)
    q_w = np.asarray(inputs["q_w"], dtype=np.float32)
    v_g = float(np.asarray(inputs["v_g"], dtype=np.float32))
    q_g = float(np.asarray(inputs["q_g"], dtype=np.float32))
    v_b = np.asarray(inputs["v_b"], dtype=np.float32)
    q_b = np.asarray(inputs["q_b"], dtype=np.float32)
    h_mat = np.asarray(inputs["h_mat"], dtype=np.float32)
    h_bias = np.asarray(inputs["h_bias"], dtype=np.float32)

    s_v = v_g / float(np.linalg.norm(v_w))
    s_q = q_g / float(np.linalg.norm(q_w))

    # weights: [K, D] -> [kb, 128(d'), t*128 + k'], then paired into
    # chunks holding kb=2c,2c+1 side by side
    wv_r = (v_w.reshape(NKB, KB, TV, 128).transpose(0, 3, 2, 1)
            .reshape(NKB, 128, TV * KB))
    wv_r = np.ascontiguousarray(
        wv_r.reshape(NKB // 2, 2, 128, TV * KB).transpose(0, 2, 1, 3)
        .reshape(NKB // 2, 128, 2 * TV * KB)).astype(bf16)
    wq_r = (q_w.reshape(NKB, KB, TQ, 128).transpose(0, 3, 2, 1)
            .reshape(NKB, 128, TQ * KB))
    wq_r = np.ascontiguousarray(
        wq_r.reshape(NKB // 2, 2, 128, TQ * KB).transpose(0, 2, 1, 3)
        .reshape(NKB // 2, 128, 2 * TQ * KB)).astype(bf16)

    hm = h_mat[0, :, 0, :]                       # [H, K]
    msb = np.ascontiguousarray(
        hm.T.reshape(NKB, 128, H).transpose(1, 0, 2).reshape(128, NKB * H))
    vb_r = np.ascontiguousarray(v_b.reshape(NKB, 128).T)
    qb_r = np.ascontiguousarray(q_b.reshape(NKB, 128).T)
    hb = h_bias[0, :, 0, 0]                      # [H]
    biasb = np.ascontiguousarray(
        np.broadcast_to(np.repeat(hb, NQ)[None, :], (128, HN)))
    ssb = np.ascontiguousarray(
        np.broadcast_to(np.array([s_v, s_q], dtype=np.float32)[None, :],
                        (128, 2)))

    in_maps = []
    for c in range(N_CORES):
        vc = v[c * B_LOC:(c + 1) * B_LOC]        # [B_LOC, NV, DV]
        qc = q[c * B_LOC:(c + 1) * B_LOC]        # [B_LOC, NQ, DQ]
        # vt: [2(half), 128(d'), t*MH + m'] with m' = (b_in_half, i)
        vt_c = np.ascontiguousarray(
            vc.reshape(2, MH, TV, 128).transpose(0, 3, 2, 1)
            .reshape(2, 128, TV * MH)).astype(bf16)
        # qt: [128(d'), t*MQ + (b, j)]
        qt_c = np.ascontiguousarray(
            qc.reshape(MQ, TQ, 128).transpose(2, 1, 0)
            .reshape(128, TQ * MQ)).astype(bf16)
        in_maps.append({
            "vt": vt_c, "qt": qt_c, "wv": wv_r, "wq": wq_r,
            "msb": msb, "vb": vb_r, "qb": qb_r, "bias": biasb, "ssb": ssb,
        })
    return in_maps


def _run(inputs, trace=False):
    if "nc" not in _CACHE:
        _CACHE["nc"] = _build_program()
    nc = _CACHE["nc"]
    in_maps = _prep_host(inputs)
    res = run_bass_kernel_spmd(nc, in_maps, list(range(N_CORES)), trace=trace)
    out = np.empty((B, H, NV, NQ), dtype=np.float32)
    for c in range(N_CORES):
        oc = np.asarray(res.results[c]["out"], dtype=np.float32)
        # oc: [2(half), 128(i), lb*HN + h*NQ + j]
        for half in range(2):
            blk = oc[half].reshape(NV, 4, H, NQ).transpose(1, 2, 0, 3)
            out[c * B_LOC + half * 4: c * B_LOC + half * 4 + 4] = blk
    return out, res


def kernel(**inputs):
    return _run(inputs)[0]


# revision 18
# speedup vs baseline: 1.0996x; 1.0996x over previous
"""Trainium2 Bass kernel for nn_BC_5274219839877.

Computes, for b=64, n_v=128, n_q=32, d_v=2048, d_q=1024, K=3072, H=8:
    v_ = relu((v_g/||v_w||) * v @ v_w^T + v_b)        [b, n_v, K]
    q_ = relu((q_g/||q_w||) * q @ q_w^T + q_b)        [b, n_q, K]
    out[b,h,i,j] = sum_k hm[h,k] v_[b,i,k] q_[b,j,k] + h_bias[h]

Sharding: data-parallel over batch across 8 NeuronCores (8 batches/core),
weights replicated.

All HBM traffic is bf16 (half the bytes of f32). The dominant v-side
matmuls run in f32r: measured on HW, f32r matmuls sustain a higher
effective issue rate (~234ns per 512-col matmul) than bf16 with FWL
weight loads (~266ns), so v-side bf16 DMA data is upconverted to f32r
on the (otherwise idle) DVE. The small q-side matmuls stay bf16 to keep
the prologue DMA light.

Structure per core (fused, nothing round-trips to DRAM):
  phase Q : all 24 k-blocks of the q-side projection -> qk_all in SBUF
  phase V0: v-side projection for batches 0-3 + bhvq contraction
  phase V1: same for batches 4-7 (wv re-streamed, 0.7MB/block of slack)
Stage-3 accumulates over all 24 k-blocks directly in PSUM (one bank per
in-flight batch; the two passes reuse the 4 banks), eliminating ~190
per-block DVE adds. The final bias-add + output DMA happen once per
pass, overlapped with the next pass.
"""

import numpy as np

import concourse.bass as bass
import concourse.tile as tile
from concourse import bacc, mybir
from concourse.bass_utils import run_bass_kernel_spmd

F32 = mybir.dt.float32
F32R = mybir.dt.float32r
BF16 = mybir.dt.bfloat16

N_CORES = 8
B = 64
B_LOC = B // N_CORES       # 8 batches per core
NV = 128
NQ = 32
DV = 2048
DQ = 1024
K = 3072
H = 8

KB = 128                   # k-block size (PSUM partition dim)
NKB = K // KB              # 24 k-blocks
TV = DV // 128             # 16 d-tiles (v side)
TQ = DQ // 128             # 8 d-tiles (q side)
MV = B_LOC * NV            # 1024 (m = (batch, i))
MQ = B_LOC * NQ            # 256  (m = (batch, j))
MH = MV // 2               # 512: m-half (4 batches) per pass
HN = H * NQ                # 256 output cols per batch
VTC = TV * MH              # 8192 cols of one vt half

WARM_N = 7                 # HAM warmup matmuls

_CACHE = {}


def _build_program():
    nc = bacc.Bacc("TRN2", target_bir_lowering=False, debug=False,
                   num_devices=N_CORES)

    qt_d = nc.dram_tensor("qt", [128, TQ * MQ], BF16, kind="ExternalInput")
    # vt: [half, quarter, 128, cols] -- quarter-granular transfers so
    # they can slip between wq chunks and upconvert incrementally
    vt_d = nc.dram_tensor("vt", [2, 4, 128, VTC // 4], BF16,
                          kind="ExternalInput")
    wq_d = nc.dram_tensor("wq", [NKB // 2, 128, 2 * TQ * KB], BF16,
                          kind="ExternalInput")
    wv_d = nc.dram_tensor("wv", [NKB, 128, TV * KB], BF16,
                          kind="ExternalInput")
    msb_d = nc.dram_tensor("msb", [128, NKB * H], F32, kind="ExternalInput")
    vb_d = nc.dram_tensor("vb", [128, NKB], F32, kind="ExternalInput")
    qb_d = nc.dram_tensor("qb", [128, NKB], F32, kind="ExternalInput")
    bias_d = nc.dram_tensor("bias", [128, HN], F32, kind="ExternalInput")
    ssb_d = nc.dram_tensor("ssb", [128, 2], F32, kind="ExternalInput")
    out_d = nc.dram_tensor("out", [2, 128, 4 * HN], BF16,
                           kind="ExternalOutput")

    relu = mybir.ActivationFunctionType.Relu

    msb = nc.alloc_sbuf_tensor("msb_s", [128, NKB * H], F32).ap()
    vb = nc.alloc_sbuf_tensor("vb_s", [128, NKB], F32).ap()
    qb = nc.alloc_sbuf_tensor("qb_s", [128, NKB], F32).ap()
    biasb = nc.alloc_sbuf_tensor("bias_s", [128, HN], F32).ap()
    ssb = nc.alloc_sbuf_tensor("ssb_s", [128, 2], F32).ap()
    warm = nc.alloc_sbuf_tensor("warm", [128, 256], BF16).ap()

    qt = nc.alloc_sbuf_tensor("qt_s", [128, TQ * MQ], BF16).ap()
    wq_s = [nc.alloc_sbuf_tensor(f"wqs{i}", [128, 2 * TQ * KB], BF16).ap()
            for i in range(6)]
    qk_all = nc.alloc_sbuf_tensor("qk_all", [128, NKB * MQ], BF16).ap()

    vt_bf = nc.alloc_sbuf_tensor("vt_bf", [128, VTC], BF16).ap()
    vt = nc.alloc_sbuf_tensor("vt_s", [128, 2 * VTC], F32R).ap()
    wv_bf = [nc.alloc_sbuf_tensor(f"wvbf{i}", [128, TV * KB], BF16).ap()
             for i in range(3)]
    wv_s = [nc.alloc_sbuf_tensor(f"wvs{i}", [128, TV * KB], F32R).ap()
            for i in range(3)]
    qx = [nc.alloc_sbuf_tensor(f"qx{i}", [128, 4 * HN], F32R).ap()
          for i in range(3)]
    vk = [nc.alloc_sbuf_tensor(f"vk{i}", [128, MH], F32R).ap()
          for i in range(3)]
    outs = nc.alloc_sbuf_tensor("outs", [128, B_LOC * HN], BF16).ap()

    psv = [nc.alloc_psum_tensor(f"psv{i}", [128, 512], F32).ap()
           for i in range(2)]
    psq = [nc.alloc_psum_tensor(f"psq{i}", [128, MQ], F32).ap()
           for i in range(2)]
    ps3 = [nc.alloc_psum_tensor(f"ps3{i}", [128, HN], F32).ap()
           for i in range(4)]

    with tile.TileContext(nc) as tc:
        # warmup tile memset on the (idle) DVE so the PE can start early
        nc.vector.memset(warm, 0.0)
        # dual-queue prologue: sync carries qt + even wq chunks + wv 0-1;
        # gpsimd carries consts + odd wq chunks + vt half-0. vt thus lands
        # mid-phase-Q, leaving the DVE time to upconvert before V0.
        nc.sync.dma_start(qt, qt_d.ap())
        nc.gpsimd.dma_start(qb, qb_d.ap())
        nc.gpsimd.dma_start(ssb, ssb_d.ap())
        nc.gpsimd.dma_start(vb, vb_d.ap())
        nc.gpsimd.dma_start(msb, msb_d.ap())
        nc.gpsimd.dma_start(biasb, bias_d.ap())
        for c in range(4):
            nc.sync.dma_start(wq_s[c], wq_d[c])

        # HAM pre-warm while the first DMAs stream
        for _ in range(WARM_N):
            nc.tensor.matmul(psv[0][:, :256], warm[:, :128], warm,
                             start=True, stop=True)

        def cvt_vt(half, sc):
            s = sc * (VTC // 2)
            nc.vector.tensor_copy(
                vt[:, half * VTC + s:half * VTC + s + VTC // 2],
                vt_bf[:, s:s + VTC // 2])

        # ---- phase Q: q-side projection, all k-blocks ----
        # wq chunk j=c+4 is prefetched at kb=2c, after its slot's previous
        # readers are emitted; vt half-0 quarters slip between late chunks
        # so everything arrives just-in-time on one queue
        for kb in range(NKB):
            c = kb // 2
            if kb % 2 == 0 and c + 4 < NKB // 2:
                j = c + 4
                nc.sync.dma_start(wq_s[j % 6], wq_d[j])
                if 7 <= j <= 10:
                    qd = j - 7      # vt quarters 0-3 after chunks 7-10
                    nc.sync.dma_start(
                        vt_bf[:, qd * 2048:(qd + 1) * 2048], vt_d[0, qd])
            if kb == 16:
                nc.sync.dma_start(wv_bf[0], wv_d[0])
                nc.sync.dma_start(wv_bf[1], wv_d[1])
            if kb == 16:
                cvt_vt(0, 0)
            if kb == 20:
                cvt_vt(0, 1)
            if kb == 22:
                nc.vector.tensor_copy(wv_s[0], wv_bf[0])
            wqb = wq_s[c % 6]
            off = (kb % 2) * TQ * KB
            ps = psq[kb % 2]
            for t in range(TQ):
                nc.tensor.matmul(
                    ps,
                    wqb[:, off + t * KB:off + (t + 1) * KB],
                    qt[:, t * MQ:(t + 1) * MQ],
                    start=(t == 0), stop=(t == TQ - 1))
            nc.scalar.activation(qk_all[:, kb * MQ:(kb + 1) * MQ], ps, relu,
                                 bias=qb[:, kb:kb + 1], scale=ssb[:, 1:2])

        def make_qx(kb, half):
            # qx[k, (b,h,j)] = hm[h,k] * qk[k, (b,j)] for this m-half's
            # 4 batches
            qxb = qx[kb % 3]
            qx4 = qxb.rearrange("p (b h j) -> p b h j", b=4, h=H)
            qk3 = qk_all[:, kb * MQ + half * 128:
                         kb * MQ + half * 128 + 128].rearrange(
                "p (b j) -> p b j", b=4)
            for h in range(H):
                nc.vector.tensor_scalar_mul(
                    qx4[:, :, h, :], qk3,
                    msb[:, kb * H + h:kb * H + h + 1])

        def stage3(kb, half):
            # out[b][i, (h,j)] += vk[:, lb].T @ qx[:, lb]; accumulates
            # in PSUM across all k-blocks, one bank per in-flight batch
            vkb = vk[kb % 3]
            qxb = qx[kb % 3]
            for lb in range(4):
                nc.tensor.matmul(
                    ps3[lb],
                    vkb[:, lb * NV:(lb + 1) * NV],
                    qxb[:, lb * HN:(lb + 1) * HN],
                    start=(kb == 0), stop=(kb == NKB - 1))

        def drain(half):
            for lb in range(4):
                b_ = half * 4 + lb
                nc.vector.tensor_add(
                    outs[:, b_ * HN:(b_ + 1) * HN], ps3[lb], biasb)
            nc.sync.dma_start(
                out_d[half],
                outs[:, half * 4 * HN:(half + 1) * 4 * HN])

        # ---- phases V0/V1: v-side projection + bhvq contraction ----
        # wv streams through a 3-deep bf16->f32r convert pipeline over a
        # global index gi = half*NKB + kb (wv re-streamed for V1):
        # at gi, DMA chunk gi+2 and upconvert chunk gi+1
        for half in range(2):
            for kb in range(NKB):
                gi = half * NKB + kb
                if gi + 2 < 2 * NKB:
                    nc.sync.dma_start(wv_bf[(gi + 2) % 3],
                                      wv_d[(gi + 2) % NKB])
                if gi + 1 < 2 * NKB:
                    nc.vector.tensor_copy(wv_s[(gi + 1) % 3],
                                          wv_bf[(gi + 1) % 3])
                if half == 0:
                    if 2 <= kb < 6:
                        qd = kb - 2
                        nc.gpsimd.dma_start(
                            vt_bf[:, qd * 2048:(qd + 1) * 2048],
                            vt_d[1, qd])
                    if kb == 6:
                        cvt_vt(1, 0)
                    if kb == 9:
                        cvt_vt(1, 1)
                if half == 1 and kb == NKB - 1:
                    break              # split tail below
                wvb = wv_s[gi % 3]
                ps = psv[kb % 2]
                for t in range(TV):
                    nc.tensor.matmul(
                        ps,
                        wvb[:, t * KB:(t + 1) * KB],
                        vt[:, half * VTC + t * MH:
                           half * VTC + t * MH + MH],
                        start=(t == 0), stop=(t == TV - 1))
                nc.scalar.activation(
                    vk[kb % 3], ps, relu,
                    bias=vb[:, kb:kb + 1], scale=ssb[:, 0:1])
                make_qx(kb, half)
                if kb >= 1:
                    stage3(kb - 1, half)
            if half == 0:
                stage3(NKB - 1, 0)
                drain(0)

        # ---- split tail: last block of V1 in two m-half-groups so the
        # activation/stage-3/drain chain overlaps the second group ----
        kb = NKB - 1
        make_qx(kb, 1)
        vkb = vk[kb % 3]
        qxb = qx[kb % 3]
        wvb = wv_s[(NKB + kb) % 3]
        for g in range(2):
            ps = psv[1 - g][:, :256]
            for t in range(TV):
                nc.tensor.matmul(
                    ps,
                    wvb[:, t * KB:(t + 1) * KB],
                    vt[:, VTC + t * MH + g * 256:
                       VTC + t * MH + g * 256 + 256],
                    start=(t == 0), stop=(t == TV - 1))
            nc.scalar.activation(
                vkb[:, g * 256:(g + 1) * 256], ps, relu,
                bias=vb[:, kb:kb + 1], scale=ssb[:, 0:1])
            if g == 0:
                stage3(kb - 1, 1)
        for g in range(2):
            for lb in (2 * g, 2 * g + 1):
                nc.tensor.matmul(
                    ps3[lb],
                    vkb[:, lb * NV:(lb + 1) * NV],
                    qxb[:, lb * HN:(lb + 1) * HN],
                    start=False, stop=True)
            for lb in (2 * g, 2 * g + 1):
                b_ = 4 + lb
                nc.vector.tensor_add(
                    outs[:, b_ * HN:(b_ + 1) * HN], ps3[lb], biasb)
            nc.sync.dma_start(
                out_d[1][:, g * 512:(g + 1) * 512],
                outs[:, (4 + 2 * g) * HN:(6 + 2 * g) * HN])

    nc.compile()
    return nc


def _prep_host(inputs):
    bf16 = mybir.dt.np(BF16)
    v = np.asarray(inputs["v"], dtype=np.float32)
    q = np.asarray(inputs["q"], dtype=np.float32)
    v_w = np.asarray(inputs["v_w"], dtype=np.float32)
    q_w = np.asarray(inputs["q_w"], dtype=np.float32)
    v_g = float(np.asarray(inputs["v_g"], dtype=np.float32))
    q_g = float(np.asarray(inputs["q_g"], dtype=np.float32))
    v_b = np.asarray(inputs["v_b"], dtype=np.float32)
    q_b = np.asarray(inputs["q_b"], dtype=np.float32)
    h_mat = np.asarray(inputs["h_mat"], dtype=np.float32)
    h_bias = np.asarray(inputs["h_bias"], dtype=np.float32)

    s_v = v_g / float(np.linalg.norm(v_w))
    s_q = q_g / float(np.linalg.norm(q_w))

    # weights: [K, D] -> [kb, 128(d'), t*128 + k']
    wv_r = np.ascontiguousarray(
        v_w.reshape(NKB, KB, TV, 128).transpose(0, 3, 2, 1)
        .reshape(NKB, 128, TV * KB)).astype(bf16)
    wq_r = (q_w.reshape(NKB, KB, TQ, 128).transpose(0, 3, 2, 1)
            .reshape(NKB, 128, TQ * KB))
    wq_r = np.ascontiguousarray(
        wq_r.reshape(NKB // 2, 2, 128, TQ * KB).transpose(0, 2, 1, 3)
        .reshape(NKB // 2, 128, 2 * TQ * KB)).astype(bf16)

    hm = h_mat[0, :, 0, :]                       # [H, K]
    msb = np.ascontiguousarray(
        hm.T.reshape(NKB, 128, H).transpose(1, 0, 2).reshape(128, NKB * H))
    vb_r = np.ascontiguousarray(v_b.reshape(NKB, 128).T)
    qb_r = np.ascontiguousarray(q_b.reshape(NKB, 128).T)
    hb = h_bias[0, :, 0, 0]                      # [H]
    biasb = np.ascontiguousarray(
        np.broadcast_to(np.repeat(hb, NQ)[None, :], (128, HN)))
    ssb = np.ascontiguousarray(
        np.broadcast_to(np.array([s_v, s_q], dtype=np.float32)[None, :],
                        (128, 2)))

    in_maps = []
    for c in range(N_CORES):
        vc = v[c * B_LOC:(c + 1) * B_LOC]        # [B_LOC, NV, DV]
        qc = q[c * B_LOC:(c + 1) * B_LOC]        # [B_LOC, NQ, DQ]
        # vt: [2(half), 4(quarter), 128(d'), cols] with cols = t*MH + m'
        vt_c = np.ascontiguousarray(
            vc.reshape(2, MH, TV, 128).transpose(0, 3, 2, 1)
            .reshape(2, 128, 4, VTC // 4).transpose(0, 2, 1, 3)
            .copy()).astype(bf16)
        # qt: [128(d'), t*MQ + (b, j)]
        qt_c = np.ascontiguousarray(
            qc.reshape(MQ, TQ, 128).transpose(2, 1, 0)
            .reshape(128, TQ * MQ)).astype(bf16)
        in_maps.append({
            "vt": vt_c, "qt": qt_c, "wv": wv_r, "wq": wq_r,
            "msb": msb, "vb": vb_r, "qb": qb_r, "bias": biasb, "ssb": ssb,
        })
    return in_maps


def _run(inputs, trace=False):
    if "nc" not in _CACHE:
        _CACHE["nc"] = _build_program()
    nc = _CACHE["nc"]
    in_maps = _prep_host(inputs)
    res = run_bass_kernel_spmd(nc, in_maps, list(range(N_CORES)), trace=trace)
    out = np.empty((B, H, NV, NQ), dtype=np.float32)
    for c in range(N_CORES):
        oc = np.asarray(res.results[c]["out"], dtype=np.float32)
        # oc: [2(half), 128(i), lb*HN + h*NQ + j]
        for half in range(2):
            blk = oc[half].reshape(NV, 4, H, NQ).transpose(1, 2, 0, 3)
            out[c * B_LOC + half * 4: c * B_LOC + half * 4 + 4] = blk
    return out, res


def kernel(**inputs):
    return _run(inputs)[0]


# revision 24
# speedup vs baseline: 1.1360x; 1.0331x over previous
"""Trainium2 Bass kernel for nn_BC_5274219839877.

Computes, for b=64, n_v=128, n_q=32, d_v=2048, d_q=1024, K=3072, H=8:
    v_ = relu((v_g/||v_w||) * v @ v_w^T + v_b)        [b, n_v, K]
    q_ = relu((q_g/||q_w||) * q @ q_w^T + q_b)        [b, n_q, K]
    out[b,h,i,j] = sum_k hm[h,k] v_[b,i,k] q_[b,j,k] + h_bias[h]

Sharding: data-parallel over batch across 8 NeuronCores (8 batches/core),
weights replicated.

All HBM traffic is bf16 (half the bytes of f32). The dominant v-side
matmuls run in f32r: measured on HW, f32r matmuls sustain a higher
effective issue rate (~234ns per 512-col matmul) than bf16 with FWL
weight loads (~266ns), so v-side bf16 DMA data is upconverted to f32r
on the (otherwise idle) DVE. The small q-side matmuls stay bf16 to keep
the prologue DMA light.

Structure per core (fused, nothing round-trips to DRAM):
  phase Q : all 24 k-blocks of the q-side projection -> qk_all in SBUF
  phase V0: v-side projection for batches 0-3 + bhvq contraction
  phase V1: same for batches 4-7 (wv re-streamed, 0.7MB/block of slack)
Stage-3 accumulates over all 24 k-blocks directly in PSUM (one bank per
in-flight batch; the two passes reuse the 4 banks), eliminating ~190
per-block DVE adds. The final bias-add + output DMA happen once per
pass, overlapped with the next pass.
"""

import numpy as np

import concourse.bass as bass
import concourse.tile as tile
from concourse import bacc, mybir
from concourse.bass_utils import run_bass_kernel_spmd

F32 = mybir.dt.float32
F32R = mybir.dt.float32r
BF16 = mybir.dt.bfloat16

N_CORES = 8
B = 64
B_LOC = B // N_CORES       # 8 batches per core
NV = 128
NQ = 32
DV = 2048
DQ = 1024
K = 3072
H = 8

KB = 128                   # k-block size (PSUM partition dim)
NKB = K // KB              # 24 k-blocks
TV = DV // 128             # 16 d-tiles (v side)
TQ = DQ // 128             # 8 d-tiles (q side)
MV = B_LOC * NV            # 1024 (m = (batch, i))
MQ = B_LOC * NQ            # 256  (m = (batch, j))
MH = MV // 2               # 512: m-half (4 batches) per pass
HN = H * NQ                # 256 output cols per batch
VTC = TV * MH              # 8192 cols of one vt half

WARM_N = 12                # HAM warmup matmuls

_CACHE = {}


def _build_program():
    nc = bacc.Bacc("TRN2", target_bir_lowering=False, debug=False,
                   num_devices=N_CORES)

    qt_d = nc.dram_tensor("qt", [128, TQ * MQ], BF16, kind="ExternalInput")
    # vt: [half, sub, 128, cols] -- two sub-chunks per half so the f32r
    # upconvert can start before the whole half lands
    vt_d = nc.dram_tensor("vt", [2, 2, 128, VTC // 2], BF16,
                          kind="ExternalInput")
    wq_d = nc.dram_tensor("wq", [NKB // 2, 128, 2 * TQ * KB], BF16,
                          kind="ExternalInput")
    wv_d = nc.dram_tensor("wv", [NKB, 128, TV * KB], BF16,
                          kind="ExternalInput")
    msb_d = nc.dram_tensor("msb", [128, NKB * H], F32, kind="ExternalInput")
    vb_d = nc.dram_tensor("vb", [128, NKB], F32, kind="ExternalInput")
    qb_d = nc.dram_tensor("qb", [128, NKB], F32, kind="ExternalInput")
    bias_d = nc.dram_tensor("bias", [128, HN], F32, kind="ExternalInput")
    ssb_d = nc.dram_tensor("ssb", [128, 2], F32, kind="ExternalInput")
    out_d = nc.dram_tensor("out", [2, 128, 4 * HN], BF16,
                           kind="ExternalOutput")

    relu = mybir.ActivationFunctionType.Relu

    msb = nc.alloc_sbuf_tensor("msb_s", [128, NKB * H], F32).ap()
    vb = nc.alloc_sbuf_tensor("vb_s", [128, NKB], F32).ap()
    qb = nc.alloc_sbuf_tensor("qb_s", [128, NKB], F32).ap()
    biasb = nc.alloc_sbuf_tensor("bias_s", [128, HN], F32).ap()
    ssb = nc.alloc_sbuf_tensor("ssb_s", [128, 2], F32).ap()
    warm = nc.alloc_sbuf_tensor("warm", [128, 256], BF16).ap()

    qt = nc.alloc_sbuf_tensor("qt_s", [128, TQ * MQ], BF16).ap()
    wq_s = [nc.alloc_sbuf_tensor(f"wqs{i}", [128, 2 * TQ * KB], BF16).ap()
            for i in range(6)]
    qk_all = nc.alloc_sbuf_tensor("qk_all", [128, NKB * MQ], BF16).ap()

    vt_bf = nc.alloc_sbuf_tensor("vt_bf", [128, VTC], BF16).ap()
    vt = nc.alloc_sbuf_tensor("vt_s", [128, 2 * VTC], F32R).ap()
    wv_bf = [nc.alloc_sbuf_tensor(f"wvbf{i}", [128, TV * KB], BF16).ap()
             for i in range(3)]
    wv_s = [nc.alloc_sbuf_tensor(f"wvs{i}", [128, TV * KB], F32R).ap()
            for i in range(3)]
    qx = [nc.alloc_sbuf_tensor(f"qx{i}", [128, 4 * HN], F32R).ap()
          for i in range(3)]
    vk = [nc.alloc_sbuf_tensor(f"vk{i}", [128, MH], F32R).ap()
          for i in range(3)]
    outs = nc.alloc_sbuf_tensor("outs", [128, B_LOC * HN], BF16).ap()

    psv = [nc.alloc_psum_tensor(f"psv{i}", [128, 512], F32).ap()
           for i in range(2)]
    psq = [nc.alloc_psum_tensor(f"psq{i}", [128, MQ], F32).ap()
           for i in range(2)]
    ps3 = [nc.alloc_psum_tensor(f"ps3{i}", [128, HN], F32).ap()
           for i in range(4)]

    with tile.TileContext(nc) as tc:
        # small persistent consts on the gpsimd queue
        nc.gpsimd.memset(warm, 0.0)
        nc.gpsimd.dma_start(qb, qb_d.ap())
        nc.gpsimd.dma_start(ssb, ssb_d.ap())
        nc.gpsimd.dma_start(vb, vb_d.ap())
        nc.gpsimd.dma_start(msb, msb_d.ap())
        nc.gpsimd.dma_start(biasb, bias_d.ap())

        # sync queue: qt + wq chunks paced against phase Q; vt half-0 and
        # the first wv blocks queued behind the last wq chunks
        nc.sync.dma_start(qt, qt_d.ap())
        nc.sync.dma_start(wq_s[0], wq_d[0])
        nc.sync.dma_start(wq_s[1], wq_d[1])
        nc.sync.dma_start(wq_s[2], wq_d[2])

        # HAM pre-warm while the first DMAs stream
        for _ in range(WARM_N):
            nc.tensor.matmul(psv[0][:, :256], warm[:, :128], warm,
                             start=True, stop=True)

        def cvt_vt(half, sc):
            s = sc * (VTC // 2)
            nc.vector.tensor_copy(
                vt[:, half * VTC + s:half * VTC + s + VTC // 2],
                vt_bf[:, s:s + VTC // 2])

        # ---- phase Q: q-side projection, all k-blocks ----
        # wq chunk j=c+3 is prefetched at kb=2c, after its slot's
        # previous readers are emitted
        for kb in range(NKB):
            c = kb // 2
            if kb % 2 == 0 and 3 <= c + 3 < NKB // 2:
                nc.sync.dma_start(wq_s[(c + 3) % 6], wq_d[c + 3])
            if kb == 16:
                nc.sync.dma_start(vt_bf[:, :VTC // 2], vt_d[0, 0])
                nc.sync.dma_start(vt_bf[:, VTC // 2:], vt_d[0, 1])
            if kb == 18:
                for i in range(3):
                    nc.sync.dma_start(wv_bf[i], wv_d[i])
                cvt_vt(0, 0)
            if kb == 22:
                cvt_vt(0, 1)
                nc.vector.tensor_copy(wv_s[0], wv_bf[0])
                nc.vector.tensor_copy(wv_s[1], wv_bf[1])
            wqb = wq_s[c % 6]
            off = (kb % 2) * TQ * KB
            ps = psq[kb % 2]
            for t in range(TQ):
                nc.tensor.matmul(
                    ps,
                    wqb[:, off + t * KB:off + (t + 1) * KB],
                    qt[:, t * MQ:(t + 1) * MQ],
                    start=(t == 0), stop=(t == TQ - 1))
            nc.scalar.activation(qk_all[:, kb * MQ:(kb + 1) * MQ], ps, relu,
                                 bias=qb[:, kb:kb + 1], scale=ssb[:, 1:2])

        def make_qx(kb, half):
            # qx[k, (b,h,j)] = hm[h,k] * qk[k, (b,j)] for this m-half's
            # 4 batches
            qxb = qx[kb % 3]
            qx4 = qxb.rearrange("p (b h j) -> p b h j", b=4, h=H)
            qk3 = qk_all[:, kb * MQ + half * 128:
                         kb * MQ + half * 128 + 128].rearrange(
                "p (b j) -> p b j", b=4)
            for h in range(H):
                nc.vector.tensor_scalar_mul(
                    qx4[:, :, h, :], qk3,
                    msb[:, kb * H + h:kb * H + h + 1])

        def stage3(kb, half):
            # out[b][i, (h,j)] += vk[:, lb].T @ qx[:, lb]; accumulates
            # in PSUM across all k-blocks, one bank per in-flight batch
            vkb = vk[kb % 3]
            qxb = qx[kb % 3]
            for lb in range(4):
                nc.tensor.matmul(
                    ps3[lb],
                    vkb[:, lb * NV:(lb + 1) * NV],
                    qxb[:, lb * HN:(lb + 1) * HN],
                    start=(kb == 0), stop=(kb == NKB - 1))

        def drain(half):
            for lb in range(4):
                b_ = half * 4 + lb
                nc.vector.tensor_add(
                    outs[:, b_ * HN:(b_ + 1) * HN], ps3[lb], biasb)
            nc.sync.dma_start(
                out_d[half],
                outs[:, half * 4 * HN:(half + 1) * 4 * HN])

        # ---- phases V0/V1: v-side projection + bhvq contraction ----
        # wv streams through a 3-deep bf16->f32r convert pipeline over a
        # global index gi = half*NKB + kb (wv re-streamed for V1):
        # at gi, DMA chunk gi+3 and upconvert chunk gi+2
        for half in range(2):
            for kb in range(NKB):
                gi = half * NKB + kb
                if gi + 3 < 2 * NKB:
                    nc.sync.dma_start(wv_bf[(gi + 3) % 3],
                                      wv_d[(gi + 3) % NKB])
                if gi + 2 < 2 * NKB:
                    nc.vector.tensor_copy(wv_s[(gi + 2) % 3],
                                          wv_bf[(gi + 2) % 3])
                if half == 0:
                    if kb == 2:
                        nc.gpsimd.dma_start(vt_bf[:, :VTC // 2], vt_d[1, 0])
                    if kb == 6:
                        nc.gpsimd.dma_start(vt_bf[:, VTC // 2:], vt_d[1, 1])
                    if kb == 4:
                        cvt_vt(1, 0)
                    if kb == 8:
                        cvt_vt(1, 1)
                if half == 1 and kb == NKB - 1:
                    break              # split tail below
                wvb = wv_s[gi % 3]
                ps = psv[kb % 2]
                for t in range(TV):
                    nc.tensor.matmul(
                        ps,
                        wvb[:, t * KB:(t + 1) * KB],
                        vt[:, half * VTC + t * MH:
                           half * VTC + t * MH + MH],
                        start=(t == 0), stop=(t == TV - 1))
                nc.scalar.activation(
                    vk[kb % 3], ps, relu,
                    bias=vb[:, kb:kb + 1], scale=ssb[:, 0:1])
                make_qx(kb, half)
                if kb >= 1:
                    stage3(kb - 1, half)
            if half == 0:
                stage3(NKB - 1, 0)
                drain(0)

        # ---- split tail: last block of V1 in two m-half-groups so the
        # activation/stage-3/drain chain overlaps the second group ----
        kb = NKB - 1
        make_qx(kb, 1)
        vkb = vk[kb % 3]
        qxb = qx[kb % 3]
        wvb = wv_s[(NKB + kb) % 3]
        for g in range(2):
            ps = psv[1 - g][:, :256]
            for t in range(TV):
                nc.tensor.matmul(
                    ps,
                    wvb[:, t * KB:(t + 1) * KB],
                    vt[:, VTC + t * MH + g * 256:
                       VTC + t * MH + g * 256 + 256],
                    start=(t == 0), stop=(t == TV - 1))
            nc.scalar.activation(
                vkb[:, g * 256:(g + 1) * 256], ps, relu,
                bias=vb[:, kb:kb + 1], scale=ssb[:, 0:1])
            if g == 0:
                stage3(kb - 1, 1)
        for g in range(2):
            for lb in (2 * g, 2 * g + 1):
                nc.tensor.matmul(
                    ps3[lb],
                    vkb[:, lb * NV:(lb + 1) * NV],
                    qxb[:, lb * HN:(lb + 1) * HN],
                    start=False, stop=True)
            for lb in (2 * g, 2 * g + 1):
                b_ = 4 + lb
                nc.vector.tensor_add(
                    outs[:, b_ * HN:(b_ + 1) * HN], ps3[lb], biasb)
            nc.sync.dma_start(
                out_d[1][:, g * 512:(g + 1) * 512],
                outs[:, (4 + 2 * g) * HN:(6 + 2 * g) * HN])

    nc.compile()
    return nc


def _prep_host(inputs):
    bf16 = mybir.dt.np(BF16)
    v = np.asarray(inputs["v"], dtype=np.float32)
    q = np.asarray(inputs["q"], dtype=np.float32)
    v_w = np.asarray(inputs["v_w"], dtype=np.float32)
    q_w = np.asarray(inputs["q_w"], dtype=np.float32)
    v_g = float(np.asarray(inputs["v_g"], dtype=np.float32))
    q_g = float(np.asarray(inputs["q_g"], dtype=np.float32))
    v_b = np.asarray(inputs["v_b"], dtype=np.float32)
    q_b = np.asarray(inputs["q_b"], dtype=np.float32)
    h_mat = np.asarray(inputs["h_mat"], dtype=np.float32)
    h_bias = np.asarray(inputs["h_bias"], dtype=np.float32)

    s_v = v_g / float(np.linalg.norm(v_w))
    s_q = q_g / float(np.linalg.norm(q_w))

    # weights: [K, D] -> [kb, 128(d'), t*128 + k']
    wv_r = np.ascontiguousarray(
        v_w.reshape(NKB, KB, TV, 128).transpose(0, 3, 2, 1)
        .reshape(NKB, 128, TV * KB)).astype(bf16)
    wq_r = (q_w.reshape(NKB, KB, TQ, 128).transpose(0, 3, 2, 1)
            .reshape(NKB, 128, TQ * KB))
    wq_r = np.ascontiguousarray(
        wq_r.reshape(NKB // 2, 2, 128, TQ * KB).transpose(0, 2, 1, 3)
        .reshape(NKB // 2, 128, 2 * TQ * KB)).astype(bf16)

    hm = h_mat[0, :, 0, :]                       # [H, K]
    msb = np.ascontiguousarray(
        hm.T.reshape(NKB, 128, H).transpose(1, 0, 2).reshape(128, NKB * H))
    vb_r = np.ascontiguousarray(v_b.reshape(NKB, 128).T)
    qb_r = np.ascontiguousarray(q_b.reshape(NKB, 128).T)
    hb = h_bias[0, :, 0, 0]                      # [H]
    biasb = np.ascontiguousarray(
        np.broadcast_to(np.repeat(hb, NQ)[None, :], (128, HN)))
    ssb = np.ascontiguousarray(
        np.broadcast_to(np.array([s_v, s_q], dtype=np.float32)[None, :],
                        (128, 2)))

    in_maps = []
    for c in range(N_CORES):
        vc = v[c * B_LOC:(c + 1) * B_LOC]        # [B_LOC, NV, DV]
        qc = q[c * B_LOC:(c + 1) * B_LOC]        # [B_LOC, NQ, DQ]
        # vt: [2(half), 2(sub), 128(d'), cols] with cols = t*MH + m'
        vt_c = np.ascontiguousarray(
            vc.reshape(2, MH, TV, 128).transpose(0, 3, 2, 1)
            .reshape(2, 128, 2, VTC // 2).transpose(0, 2, 1, 3)
            .copy()).astype(bf16)
        # qt: [128(d'), t*MQ + (b, j)]
        qt_c = np.ascontiguousarray(
            qc.reshape(MQ, TQ, 128).transpose(2, 1, 0)
            .reshape(128, TQ * MQ)).astype(bf16)
        in_maps.append({
            "vt": vt_c, "qt": qt_c, "wv": wv_r, "wq": wq_r,
            "msb": msb, "vb": vb_r, "qb": qb_r, "bias": biasb, "ssb": ssb,
        })
    return in_maps


def _run(inputs, trace=False):
    if "nc" not in _CACHE:
        _CACHE["nc"] = _build_program()
    nc = _CACHE["nc"]
    in_maps = _prep_host(inputs)
    res = run_bass_kernel_spmd(nc, in_maps, list(range(N_CORES)), trace=trace)
    out = np.empty((B, H, NV, NQ), dtype=np.float32)
    for c in range(N_CORES):
        oc = np.asarray(res.results[c]["out"], dtype=np.float32)
        # oc: [2(half), 128(i), lb*HN + h*NQ + j]
        for half in range(2):
            blk = oc[half].reshape(NV, 4, H, NQ).transpose(1, 2, 0, 3)
            out[c * B_LOC + half * 4: c * B_LOC + half * 4 + 4] = blk
    return out, res


def kernel(**inputs):
    return _run(inputs)[0]
